# revision 46
# baseline (speedup 1.0000x reference)
"""Multi-head latent attention (MLA prefill) Trainium2 kernel.

Contract: kernel(**inputs) takes the FULL unsharded inputs (np arrays, keyed as
setup_inputs()) and returns the full outputs (output, new_c_kv, new_k_rope).

Sharding: 8 cores = 2 batches x 4 head-groups (4 heads each).  Each core:
  - recomputes the small low-rank A-projections + RMS norms for its batch
    (replicated inside the batch group),
  - computes q_b / kv_b / attention / o_proj for its 4 heads,
  - writes a partial o_proj output (summed over the 4 head-group cores on host)
    plus c_kv^T / k_rope^T aux outputs (read from one core per batch).

All on-device activations are FEATURE-major (hidden is pre-transposed on the
host) so the kernel needs no on-device transposes:
  - matmul(out[M,N], lhsT[K,M], rhs[K,N]) contracts partition dim K,
  - feature-major out  <- lhsT = weights,            rhs = act^T
  - token-major   out  <- lhsT = act^T chunk,        rhs = weights
  - attention: S^T[s,q] = K^T slices as lhsT, Q^T as rhs; P^T = exp(S^T);
    O^T[v,q] accumulates lhsT=V[s,v] token-major, rhs=P^T; O^T then feeds
    o_proj directly as the stationary operand.
Softmax skips the running max (scores are O(1) here; exp stays in fp32 range)
and uses l = ones^T @ P^T (a PE partition-sum) as the denominator.  RMS norm
stats use the same ones-matmul trick; per-token inverse scales are broadcast
across partitions with a rank-1 PE outer product.
"""

import numpy as np
import ml_dtypes

import concourse.bacc as bacc
import concourse.mybir as mybir
import concourse.tile as tile
from concourse.bass_utils import run_bass_kernel_spmd

# Problem dims (hardcoded per the harness contract)
B, T, D = 2, 2048, 2048
H = 16
NOPE, ROPE = 128, 64
Q_HEAD = NOPE + ROPE          # 192
Q_LORA = 768
KV_RANK = 512
V_DIM = 128
EPS = 1e-6
SCALE = Q_HEAD ** -0.5

P = 128                       # partitions
TG = 512                      # token group (free-dim tile)
NG = T // TG                  # 4 groups
KD = D // P                   # 16 hidden chunks
QLC = Q_LORA // P             # 6 q_lora chunks
KVC = KV_RANK // P            # 4 rank chunks
HL = 4                        # local heads per core
RH = ROPE // 2                # 32

F32 = mybir.dt.float32
F32R = mybir.dt.float32r
BF16 = mybir.dt.bfloat16

_PROG = None


def _emit(nc, tc, io):
    from contextlib import ExitStack
    ctx = ExitStack()
    with ctx:
        _emit_body(nc, tc, io, ctx)


def _emit_body(nc, tc, io, ctx):
    (hT, wqa, wkva, wqbn, wqbr, wkvbn, wkvbv, wo, gk_d, cosq, sinq, cosk, sink,
     masks_d, ones_r_d, o_out, ckv_out, kro_out) = io

    const = ctx.enter_context(tc.tile_pool(name="const", bufs=1))
    res = ctx.enter_context(tc.tile_pool(name="res", bufs=1))
    sb = ctx.enter_context(tc.tile_pool(name="sb", bufs=1))
    ps = ctx.enter_context(tc.tile_pool(name="ps", bufs=1, space="PSUM"))

    def stile(shape, dt, tag, bufs):
        return sb.tile(shape, dt, tag=tag, bufs=bufs, name=tag)

    def ptile(shape, tag, bufs):
        return ps.tile(shape, F32, tag=tag, bufs=bufs, name=tag)

    MM = ("mm", 3)       # short-lived matmul outputs
    PA = ("pa", 2)       # long A-proj accumulation chains
    ACC = ("accum", 3)   # accumulators alive across a loop (ss / l / oT)

    # ---------------- startup-critical loads first ----------------
    # interleave hid(g0) with wqa so the first A-proj chain starts immediately
    hid0 = []
    wqa_t, wkva_t = [], []
    for k in range(KD):
        t = stile([P, TG], BF16, "hid", KD + 3)
        nc.sync.dma_start(t[:], hT[k * P:(k + 1) * P, 0:TG])
        hid0.append(t)
        t = res.tile([P, Q_LORA], BF16, tag=f"wqa{k}", name=f"wqa{k}")
        nc.sync.dma_start(t[:], wqa[k * P:(k + 1) * P, :])
        wqa_t.append(t)
    for k in range(KD):
        t = res.tile([P, KV_RANK + ROPE], BF16, tag=f"wkva{k}", name=f"wkva{k}")
        nc.sync.dma_start(t[:], wkva[k * P:(k + 1) * P, :])
        wkva_t.append(t)

    # ---------------- small constants ----------------
    ones_l = const.tile([P, 1], BF16, tag="ones_l")
    nc.vector.memset(ones_l[:], 1.0)
    ones_r = const.tile([1, P], F32R, tag="ones_r")
    nc.sync.dma_start(ones_r[:], ones_r_d[:])
    eps_t = const.tile([1, 1], F32, tag="eps_t")
    nc.vector.memset(eps_t[:], EPS)
    gk = []
    for m in range(KVC):
        g = const.tile([P, 1], F32, tag=f"gk{m}")
        nc.sync.dma_start(g[:], gk_d[m * P:(m + 1) * P, :])
        gk.append(g)
    masks = []
    for k in range(4):
        m = const.tile([P, TG], BF16, tag=f"mask{k}")
        nc.sync.dma_start(m[:], masks_d[k])
        masks.append(m)

    tab_dram = {"cq": cosq, "sq": sinq, "ck": cosk, "sk": sink}

    # ---------------- resident weights ----------------
    wqbn_t, wqbr_t = [], []
    for k in range(QLC):
        t = res.tile([P, HL * NOPE], BF16, tag=f"wqbn{k}")
        nc.sync.dma_start(t[:], wqbn[k * P:(k + 1) * P, :])
        wqbn_t.append(t)
        t = res.tile([P, HL * ROPE], BF16, tag=f"wqbr{k}")
        nc.sync.dma_start(t[:], wqbr[k * P:(k + 1) * P, :])
        wqbr_t.append(t)
    wkvbn_t, wkvbv_t = [], []
    for k in range(KVC):
        t = res.tile([P, HL * NOPE], BF16, tag=f"wkvbn{k}")
        nc.sync.dma_start(t[:], wkvbn[k * P:(k + 1) * P, :])
        wkvbn_t.append(t)
        t = res.tile([P, HL * V_DIM], BF16, tag=f"wkvbv{k}")
        nc.sync.dma_start(t[:], wkvbv[k * P:(k + 1) * P, :])
        wkvbv_t.append(t)

    wo_t = []
    for h in range(HL):
        t = res.tile([P, D], BF16, tag=f"wo{h}", name=f"wo{h}")
        nc.sync.dma_start(t[:], wo[h * P:(h + 1) * P, :])
        wo_t.append(t)

    # ---------------- resident activations ----------------
    knT = [res.tile([P, T], BF16, tag=f"knT{h}", name=f"knT{h}") for h in range(HL)]
    krotT = res.tile([ROPE, T], BF16, tag="krotT")
    V = [res.tile([P, HL * V_DIM], BF16, tag=f"V{t}", name=f"V{t}") for t in range(T // P)]

    def load_tab(name, g):
        t = stile([ROPE, TG], BF16, f"tabg_{name}", 1)
        nc.sync.dma_start(t[:], tab_dram[name][:, g * TG:(g + 1) * TG])
        return t

    def rope_rotate(src_ap, cos_t, sin_t, out_ap):
        """src_ap: [64, TG] fp32 (sbuf or psum); cos/sin: [64, TG] bf16."""
        t1 = stile([ROPE, TG], F32, "rope_t1", 1)
        t2 = stile([ROPE, TG], F32, "rope_t2", 1)
        nc.vector.tensor_mul(t1[:], src_ap, cos_t[:])
        nc.vector.tensor_mul(t2[0:RH, :], src_ap[RH:ROPE, :], sin_t[0:RH, :])
        nc.vector.tensor_mul(t2[RH:ROPE, :], src_ap[0:RH, :], sin_t[RH:ROPE, :])
        nc.vector.tensor_add(out_ap, t1[:], t2[:])

    # ================= per token-group pipeline =================
    for g in range(NG):
        gsl = slice(g * TG, (g + 1) * TG)

        # ---- hidden^T chunks for this group ----
        if g == 0:
            hid = hid0
        else:
            hid = []
            for k in range(KD):
                t = stile([P, TG], BF16, "hid", KD + 3)
                nc.sync.dma_start(t[:], hT[k * P:(k + 1) * P, gsl])
                hid.append(t)

        # ---- A-proj: q side (evict each chunk to SBUF promptly) ----
        qlo = []
        for m in range(QLC):
            pa = ptile([P, TG], *PA)
            for k in range(KD):
                nc.tensor.matmul(pa[:], wqa_t[k][:, m * P:(m + 1) * P], hid[k][:],
                                 start=(k == 0), stop=(k == KD - 1))
            ql = stile([P, TG], F32, "alo", 7)
            nc.scalar.copy(ql[:], pa[:])
            qlo.append(ql)

        # ---- RMS stats q ----
        ss_q = ptile([1, TG], *ACC)
        for m in range(QLC):
            sq = stile([P, TG], BF16, "sq", 1)
            nc.scalar.square(sq[:], qlo[m][:])
            nc.tensor.matmul(ss_q[:], ones_l[:], sq[:],
                             start=(m == 0), stop=(m == QLC - 1))
        rsq_q = stile([1, TG], F32, "rsq", 1)
        nc.scalar.activation(rsq_q[:], ss_q[:], mybir.ActivationFunctionType.Sqrt,
                             bias=eps_t[:], scale=1.0 / Q_LORA)
        inv_q = stile([1, TG], F32, "inv", 2)
        nc.vector.reciprocal(inv_q[:], rsq_q[:])
        bcq = stile([P, TG], F32, "bc", 2)
        nc.gpsimd.partition_broadcast(bcq[:], inv_q[:])

        qrms = []
        for m in range(QLC):
            qt = stile([P, TG], BF16, "qrms", QLC + 1)
            nc.vector.tensor_mul(qt[:], qlo[m][:], bcq[:])
            qrms.append(qt)

        # ---- A-proj: kv side ----
        kvlo = []
        for m in range(KVC):
            pa = ptile([P, TG], *PA)
            for k in range(KD):
                nc.tensor.matmul(pa[:], wkva_t[k][:, m * P:(m + 1) * P], hid[k][:],
                                 start=(k == 0), stop=(k == KD - 1))
            kl = stile([P, TG], F32, "alo", 7)
            nc.scalar.copy(kl[:], pa[:])
            kvlo.append(kl)
        # rope chunk [64, TG]
        krp = ptile([ROPE, TG], *PA)
        for k in range(KD):
            nc.tensor.matmul(krp[:], wkva_t[k][:, KV_RANK:KV_RANK + ROPE], hid[k][:],
                             start=(k == 0), stop=(k == KD - 1))
        kro = stile([ROPE, TG], F32, "kro", 1)
        nc.scalar.copy(kro[:], krp[:])
        nc.sync.dma_start(kro_out[:, gsl], kro[:])
        ck_g, sk_g = load_tab("ck", g), load_tab("sk", g)
        rope_rotate(krp[:], ck_g, sk_g, krotT[:, gsl])

        # ---- RMS stats kv ----
        ss_kv = ptile([1, TG], *ACC)
        for m in range(KVC):
            sq = stile([P, TG], BF16, "sq", 1)
            nc.scalar.square(sq[:], kvlo[m][:])
            nc.tensor.matmul(ss_kv[:], ones_l[:], sq[:],
                             start=(m == 0), stop=(m == KVC - 1))
        rsq_kv = stile([1, TG], F32, "rsq", 1)
        nc.scalar.activation(rsq_kv[:], ss_kv[:], mybir.ActivationFunctionType.Sqrt,
                             bias=eps_t[:], scale=1.0 / KV_RANK)
        inv_kv = stile([1, TG], F32, "inv", 2)
        nc.vector.reciprocal(inv_kv[:], rsq_kv[:])
        bckv = stile([P, TG], F32, "bc", 2)
        nc.gpsimd.partition_broadcast(bckv[:], inv_kv[:])

        # ---- c_kv^T = rms(kvc)*g : bf16 for compute + fp32 for output ----
        ckvg = []
        for m in range(KVC):
            cb = stile([P, TG], BF16, "ckvg", KVC + 1)
            nc.vector.scalar_tensor_tensor(
                cb[:], kvlo[m][:], gk[m][:], bckv[:],
                op0=mybir.AluOpType.mult, op1=mybir.AluOpType.mult)
            ckvg.append(cb)
            co = stile([P, TG], F32, "ckvo", 1)
            nc.vector.scalar_tensor_tensor(
                co[:], kvlo[m][:], gk[m][:], bckv[:],
                op0=mybir.AluOpType.mult, op1=mybir.AluOpType.mult)
            nc.sync.dma_start(ckv_out[m * P:(m + 1) * P, gsl], co[:])

        # ---- q_b projection for this group ----
        qnT = []
        for m in range(HL):
            pq = ptile([P, TG], *MM)
            for k in range(QLC):
                nc.tensor.matmul(pq[:], wqbn_t[k][:, m * P:(m + 1) * P], qrms[k][:],
                                 start=(k == 0), stop=(k == QLC - 1))
            qt = stile([P, TG], BF16, "qnT", HL + 1)
            nc.scalar.copy(qt[:], pq[:])
            qnT.append(qt)
        qrT = []
        cq_g, sq_g = load_tab("cq", g), load_tab("sq", g)
        for mr in range(2):   # two rope chunks, 2 heads each
            pq = ptile([P, TG], *MM)
            for k in range(QLC):
                nc.tensor.matmul(pq[:], wqbr_t[k][:, mr * P:(mr + 1) * P], qrms[k][:],
                                 start=(k == 0), stop=(k == QLC - 1))
            for half in range(2):
                qt = stile([ROPE, TG], BF16, "qrT", 4)
                rope_rotate(pq[half * ROPE:(half + 1) * ROPE, :],
                            cq_g, sq_g, qt[:])
                qrT.append(qt)

        # ---- kv_b projection for this group ----
        for h in range(HL):
            pk = ptile([P, TG], *MM)
            for k in range(KVC):
                nc.tensor.matmul(pk[:], wkvbn_t[k][:, h * P:(h + 1) * P], ckvg[k][:],
                                 start=(k == 0), stop=(k == KVC - 1))
            nc.scalar.copy(knT[h][:, gsl], pk[:])
        for t in range(TG // P):
            pv = ptile([P, HL * V_DIM], *MM)
            for k in range(KVC):
                nc.tensor.matmul(pv[:], ckvg[k][:, t * P:(t + 1) * P], wkvbv_t[k][:],
                                 start=(k == 0), stop=(k == KVC - 1))
            nc.scalar.copy(V[g * (TG // P) + t][:], pv[:])

        # ---- attention: this group's queries, all 4 local heads ----
        onorm = []
        for h in range(HL):
            jmax = 4 * g + 3
            l_p = ptile([1, TG], *ACC)
            o_p = ptile([P, TG], *ACC)
            for j in range(jmax + 1):
                off = max(0, (j - 4 * g) * P)   # masked q-columns before off
                osl = slice(off, TG)
                s_p = ptile([P, TG], *MM)
                nc.tensor.matmul(s_p[:, osl], knT[h][:, j * P:(j + 1) * P],
                                 qnT[h][:, osl], start=True, stop=False)
                nc.tensor.matmul(s_p[:, osl], krotT[:, j * P:(j + 1) * P],
                                 qrT[h][:, osl], start=False, stop=True)
                pt = stile([P, TG], BF16, "PT", 4)
                nc.scalar.activation(pt[:, osl], s_p[:, osl],
                                     mybir.ActivationFunctionType.Exp)
                if j > 4 * g:
                    nc.vector.tensor_mul(pt[:, osl], pt[:, osl],
                                         masks[j - 4 * g][:, osl])
                elif j == 4 * g:
                    nc.vector.tensor_mul(pt[:], pt[:], masks[0][:])
                nc.tensor.matmul(l_p[:, osl], ones_l[:], pt[:, osl],
                                 start=(j == 0), stop=(j == jmax),
                                 skip_group_check=True)
                nc.tensor.matmul(o_p[:, osl], V[j][:, h * V_DIM:(h + 1) * V_DIM],
                                 pt[:, osl], start=(j == 0), stop=(j == jmax),
                                 skip_group_check=True)
            invl = stile([1, TG], F32, "inv", 2)
            nc.vector.reciprocal(invl[:], l_p[:])
            bcl = stile([P, TG], F32, "bc", 2)
            nc.gpsimd.partition_broadcast(bcl[:], invl[:])
            ot = stile([P, TG], BF16, "onorm", HL + 1)
            nc.vector.tensor_mul(ot[:], o_p[:], bcl[:])
            onorm.append(ot)

        # ---- o_proj (partial over local heads) for this group's tokens ----
        for dn in range(D // TG):
            for t in range(TG // P):
                po = ptile([P, TG], *MM)
                for h in range(HL):
                    nc.tensor.matmul(po[:], onorm[h][:, t * P:(t + 1) * P],
                                     wo_t[h][:, dn * TG:(dn + 1) * TG],
                                     start=(h == 0), stop=(h == HL - 1))
                oe = stile([P, TG], F32, "oe", 2)
                nc.vector.tensor_copy(oe[:], po[:])
                row = (g * (TG // P) + t) * P
                nc.sync.dma_start(o_out[row:row + P, dn * TG:(dn + 1) * TG], oe[:])


def _build_program():
    nc = bacc.Bacc("TRN2", target_bir_lowering=False, debug=False, num_devices=8)
    io = (
        nc.dram_tensor("hT", [D, T], BF16, kind="ExternalInput").ap(),
        nc.dram_tensor("wqa", [D, Q_LORA], BF16, kind="ExternalInput").ap(),
        nc.dram_tensor("wkva", [D, KV_RANK + ROPE], BF16, kind="ExternalInput").ap(),
        nc.dram_tensor("wqbn", [Q_LORA, HL * NOPE], BF16, kind="ExternalInput").ap(),
        nc.dram_tensor("wqbr", [Q_LORA, HL * ROPE], BF16, kind="ExternalInput").ap(),
        nc.dram_tensor("wkvbn", [KV_RANK, HL * NOPE], BF16, kind="ExternalInput").ap(),
        nc.dram_tensor("wkvbv", [KV_RANK, HL * V_DIM], BF16, kind="ExternalInput").ap(),
        nc.dram_tensor("wo", [HL * V_DIM, D], BF16, kind="ExternalInput").ap(),
        nc.dram_tensor("gk", [KV_RANK, 1], F32, kind="ExternalInput").ap(),
        nc.dram_tensor("cosq", [ROPE, T], BF16, kind="ExternalInput").ap(),
        nc.dram_tensor("sinq", [ROPE, T], BF16, kind="ExternalInput").ap(),
        nc.dram_tensor("cosk", [ROPE, T], BF16, kind="ExternalInput").ap(),
        nc.dram_tensor("sink", [ROPE, T], BF16, kind="ExternalInput").ap(),
        nc.dram_tensor("masks", [4, P, TG], BF16, kind="ExternalInput").ap(),
        nc.dram_tensor("ones_r", [1, P], F32R, kind="ExternalInput").ap(),
        nc.dram_tensor("o_part", [T, D], F32, kind="ExternalOutput").ap(),
        nc.dram_tensor("ckv_T", [KV_RANK, T], F32, kind="ExternalOutput").ap(),
        nc.dram_tensor("kro_T", [ROPE, T], F32, kind="ExternalOutput").ap(),
    )
    with tile.TileContext(nc) as tc:
        _emit(nc, tc, io)
    nc.compile()
    return nc


def _get_program():
    global _PROG
    if _PROG is None:
        _PROG = _build_program()
    return _PROG


def _rope_tables_np(pos):
    """pos: int array [T] -> (cos^T, sinN^T) [64, T] fp32, sin sign-folded."""
    inv_freq = (1.0 / (10000.0 ** (np.arange(0, ROPE, 2, dtype=np.float32) / ROPE)))
    freqs = pos.astype(np.float32)[:, None] * inv_freq[None, :].astype(np.float32)
    emb = np.concatenate([freqs, freqs], axis=-1)                      # [T, 64]
    cos = np.cos(emb).astype(np.float32)
    sin = np.sin(emb).astype(np.float32)
    sinN = sin.copy()
    sinN[:, :RH] = -sinN[:, :RH]
    return np.ascontiguousarray(cos.T), np.ascontiguousarray(sinN.T)


def _prepare_in_maps(inputs):
    hidden = np.asarray(inputs["hidden_states"], dtype=np.float32)
    positions = np.asarray(inputs["positions"])
    w_qa = np.asarray(inputs["w_qa"], dtype=np.float32)
    g_qa = np.asarray(inputs["g_qa"], dtype=np.float32)
    w_qb = np.asarray(inputs["w_qb"], dtype=np.float32)
    w_kva = np.asarray(inputs["w_kva"], dtype=np.float32)
    g_kva = np.asarray(inputs["g_kva"], dtype=np.float32)
    w_kvb = np.asarray(inputs["w_kvb"], dtype=np.float32)
    w_o = np.asarray(inputs["w_o"], dtype=np.float32)

    bf = ml_dtypes.bfloat16
    # fold g_qa (rows) + attention scale into w_qb; split nope/rope per head
    wqb_f = (g_qa[:, None] * w_qb * np.float32(SCALE)).reshape(Q_LORA, H, Q_HEAD)
    wkvb_r = w_kvb.reshape(KV_RANK, H, NOPE + V_DIM)
    wo_r = w_o.reshape(H, V_DIM, D)

    # masks: mask_k[x, y] = 1 where y - x - 128k >= 0
    yy = np.arange(TG)[None, :]
    xx = np.arange(P)[:, None]
    masks = np.stack([(yy - xx - P * k >= 0) for k in range(4)]).astype(bf)

    cosk_t, sink_t = _rope_tables_np(np.arange(T))
    cosk_b, sink_b = cosk_t.astype(bf), sink_t.astype(bf)

    in_maps = []
    for c in range(8):
        b, hg = divmod(c, 4)
        hs = slice(hg * HL, (hg + 1) * HL)
        cosq_t, sinq_t = _rope_tables_np(np.asarray(positions[b]))
        in_maps.append({
            "hT": np.ascontiguousarray(hidden[b].T).astype(bf),
            "wqa": w_qa.astype(bf),
            "wkva": w_kva.astype(bf),
            "wqbn": np.ascontiguousarray(
                wqb_f[:, hs, :NOPE].reshape(Q_LORA, HL * NOPE)).astype(bf),
            "wqbr": np.ascontiguousarray(
                wqb_f[:, hs, NOPE:].reshape(Q_LORA, HL * ROPE)).astype(bf),
            "wkvbn": np.ascontiguousarray(
                wkvb_r[:, hs, :NOPE].reshape(KV_RANK, HL * NOPE)).astype(bf),
            "wkvbv": np.ascontiguousarray(
                wkvb_r[:, hs, NOPE:].reshape(KV_RANK, HL * V_DIM)).astype(bf),
            "wo": np.ascontiguousarray(wo_r[hs].reshape(HL * V_DIM, D)).astype(bf),
            "gk": g_kva.reshape(KV_RANK, 1),
            "cosq": cosq_t.astype(bf),
            "sinq": sinq_t.astype(bf),
            "cosk": cosk_b,
            "sink": sink_b,
            "masks": masks,
            "ones_r": np.ones((1, P), np.float32),
        })
    return in_maps


def kernel(**inputs):
    nc = _get_program()
    in_maps = _prepare_in_maps(inputs)
    res = run_bass_kernel_spmd(nc, in_maps, list(range(8))).results

    output = np.empty((B, T, D), np.float32)
    new_c_kv = np.empty((B, T, KV_RANK), np.float32)
    new_k_rope = np.empty((B, T, ROPE), np.float32)
    for b in range(B):
        output[b] = (res[4 * b]["o_part"] + res[4 * b + 1]["o_part"]
                     + res[4 * b + 2]["o_part"] + res[4 * b + 3]["o_part"])
        new_c_kv[b] = res[4 * b]["ckv_T"].T
        new_k_rope[b] = res[4 * b]["kro_T"].T
    return output, new_c_kv, new_k_rope


# revision 58
# speedup vs baseline: 1.0301x; 1.0301x over previous
"""Multi-head latent attention (MLA prefill) Trainium2 kernel.

Contract: kernel(**inputs) takes the FULL unsharded inputs (np arrays, keyed as
setup_inputs()) and returns the full outputs (output, new_c_kv, new_k_rope).

Sharding: 8 cores = 2 batches x 4 head-groups (4 heads each).  Each core:
  - recomputes the small low-rank A-projections + RMS norms for its batch
    (replicated inside the batch group),
  - computes q_b / kv_b / attention / o_proj for its 4 heads,
  - writes a partial o_proj output (summed over the 4 head-group cores on host)
    plus c_kv^T / k_rope^T aux outputs (read from one core per batch).

All on-device activations are FEATURE-major (hidden is pre-transposed on the
host) so the kernel needs no on-device transposes:
  - matmul(out[M,N], lhsT[K,M], rhs[K,N]) contracts partition dim K,
  - feature-major out  <- lhsT = weights,            rhs = act^T
  - token-major   out  <- lhsT = act^T chunk,        rhs = weights
  - attention: S^T[s,q] = K^T slices as lhsT, Q^T as rhs; P^T = exp(S^T);
    O^T[v,q] accumulates lhsT=V[s,v] token-major, rhs=P^T; O^T then feeds
    o_proj directly as the stationary operand.
Softmax skips the running max (scores are O(1) here; exp stays in fp32 range)
and uses l = ones^T @ P^T (a PE partition-sum) as the denominator.  RMS norm
stats use the same ones-matmul trick; per-token inverse scales are broadcast
across partitions with a rank-1 PE outer product.
"""

import numpy as np
import ml_dtypes

import concourse.bacc as bacc
import concourse.mybir as mybir
import concourse.tile as tile
from concourse.bass_utils import run_bass_kernel_spmd

# Problem dims (hardcoded per the harness contract)
B, T, D = 2, 2048, 2048
H = 16
NOPE, ROPE = 128, 64
Q_HEAD = NOPE + ROPE          # 192
Q_LORA = 768
KV_RANK = 512
V_DIM = 128
EPS = 1e-6
SCALE = Q_HEAD ** -0.5

P = 128                       # partitions
TG = 512                      # token group (free-dim tile)
NG = T // TG                  # 4 groups
KD = D // P                   # 16 hidden chunks
QLC = Q_LORA // P             # 6 q_lora chunks
KVC = KV_RANK // P            # 4 rank chunks
HL = 4                        # local heads per core
RH = ROPE // 2                # 32

F32 = mybir.dt.float32
F32R = mybir.dt.float32r
BF16 = mybir.dt.bfloat16

_PROG = None


def _emit(nc, tc, io):
    from contextlib import ExitStack
    ctx = ExitStack()
    with ctx:
        _emit_body(nc, tc, io, ctx)


def _emit_body(nc, tc, io, ctx):
    (hT, wqa, wkva, wqbn, wqbr, wkvbn, wkvbv, wo, gk_d, cosq, sinq, cosk, sink,
     masks_d, ones_r_d, o_out, ckv_out, kro_out) = io

    const = ctx.enter_context(tc.tile_pool(name="const", bufs=1))
    res = ctx.enter_context(tc.tile_pool(name="res", bufs=1))
    sb = ctx.enter_context(tc.tile_pool(name="sb", bufs=1))
    ps = ctx.enter_context(tc.tile_pool(name="ps", bufs=1, space="PSUM"))

    def stile(shape, dt, tag, bufs):
        return sb.tile(shape, dt, tag=tag, bufs=bufs, name=tag)

    def ptile(shape, tag, bufs):
        return ps.tile(shape, F32, tag=tag, bufs=bufs, name=tag)

    MM = ("mm", 3)       # short-lived matmul outputs
    PA = ("pa", 2)       # long A-proj accumulation chains
    ACC = ("accum", 3)   # accumulators alive across a loop (ss / l / oT)

    # ---------------- startup-critical loads first ----------------
    # interleave hid(g0) with wqa so the first A-proj chain starts immediately
    hid0 = []
    wqa_t, wkva_t = [], []
    for k in range(KD):
        t = stile([P, TG], BF16, "hid", KD + 3)
        nc.scalar.dma_start(t[:], hT[k * P:(k + 1) * P, 0:TG])
        hid0.append(t)
        t = res.tile([P, Q_LORA], BF16, tag=f"wqa{k}", name=f"wqa{k}")
        nc.sync.dma_start(t[:], wqa[k * P:(k + 1) * P, :])
        wqa_t.append(t)
    for k in range(KD):
        t = res.tile([P, KV_RANK + ROPE], BF16, tag=f"wkva{k}", name=f"wkva{k}")
        nc.sync.dma_start(t[:], wkva[k * P:(k + 1) * P, :])
        wkva_t.append(t)

    # ---------------- small constants ----------------
    ones_l = const.tile([P, 1], BF16, tag="ones_l")
    nc.vector.memset(ones_l[:], 1.0)
    ones_r = const.tile([1, P], F32R, tag="ones_r")
    nc.sync.dma_start(ones_r[:], ones_r_d[:])
    eps_t = const.tile([1, 1], F32, tag="eps_t")
    nc.vector.memset(eps_t[:], EPS)
    gk = []
    for m in range(KVC):
        g = const.tile([P, 1], F32, tag=f"gk{m}")
        nc.sync.dma_start(g[:], gk_d[m * P:(m + 1) * P, :])
        gk.append(g)
    masks = []
    for k in range(4):
        m = const.tile([P, TG], BF16, tag=f"mask{k}")
        nc.sync.dma_start(m[:], masks_d[k])
        masks.append(m)

    tab_dram = {"cq": cosq, "sq": sinq, "ck": cosk, "sk": sink}

    # ---------------- resident weights ----------------
    wqbn_t, wqbr_t = [], []
    for k in range(QLC):
        t = res.tile([P, HL * NOPE], BF16, tag=f"wqbn{k}")
        nc.sync.dma_start(t[:], wqbn[k * P:(k + 1) * P, :])
        wqbn_t.append(t)
        t = res.tile([P, HL * ROPE], BF16, tag=f"wqbr{k}")
        nc.sync.dma_start(t[:], wqbr[k * P:(k + 1) * P, :])
        wqbr_t.append(t)
    wkvbn_t, wkvbv_t = [], []
    for k in range(KVC):
        t = res.tile([P, HL * NOPE], BF16, tag=f"wkvbn{k}")
        nc.sync.dma_start(t[:], wkvbn[k * P:(k + 1) * P, :])
        wkvbn_t.append(t)
        t = res.tile([P, HL * V_DIM], BF16, tag=f"wkvbv{k}")
        nc.sync.dma_start(t[:], wkvbv[k * P:(k + 1) * P, :])
        wkvbv_t.append(t)

    wo_t = []
    for h in range(HL):
        t = res.tile([P, D], BF16, tag=f"wo{h}", name=f"wo{h}")
        nc.sync.dma_start(t[:], wo[h * P:(h + 1) * P, :])
        wo_t.append(t)

    # ---------------- resident activations ----------------
    knT = [res.tile([P, T], BF16, tag=f"knT{h}", name=f"knT{h}") for h in range(HL)]
    krotT = res.tile([ROPE, T], BF16, tag="krotT")
    V = [res.tile([P, HL * V_DIM], BF16, tag=f"V{t}", name=f"V{t}") for t in range(T // P)]

    def load_tab(name, g):
        t = stile([ROPE, TG], BF16, f"tabg_{name}", 1)
        nc.sync.dma_start(t[:], tab_dram[name][:, g * TG:(g + 1) * TG])
        return t

    def rope_rotate(src_ap, cos_t, sin_t, out_ap):
        """src_ap: [64, TG] fp32 (sbuf or psum); cos/sin: [64, TG] bf16."""
        t1 = stile([ROPE, TG], F32, "rope_t1", 1)
        t2 = stile([ROPE, TG], F32, "rope_t2", 1)
        nc.vector.tensor_mul(t1[:], src_ap, cos_t[:])
        nc.vector.tensor_mul(t2[0:RH, :], src_ap[RH:ROPE, :], sin_t[0:RH, :])
        nc.vector.tensor_mul(t2[RH:ROPE, :], src_ap[0:RH, :], sin_t[RH:ROPE, :])
        nc.vector.tensor_add(out_ap, t1[:], t2[:])

    # ================= per token-group pipeline =================
    for g in range(NG):
        gsl = slice(g * TG, (g + 1) * TG)

        # ---- hidden^T chunks for this group ----
        if g == 0:
            hid = hid0
        else:
            hid = []
            for k in range(KD):
                t = stile([P, TG], BF16, "hid", KD + 3)
                nc.scalar.dma_start(t[:], hT[k * P:(k + 1) * P, gsl])
                hid.append(t)

        # ---- A-proj: q side (evict each chunk to SBUF promptly) ----
        qlo = []
        for m in range(QLC):
            pa = ptile([P, TG], *PA)
            for k in range(KD):
                nc.tensor.matmul(pa[:], wqa_t[k][:, m * P:(m + 1) * P], hid[k][:],
                                 start=(k == 0), stop=(k == KD - 1))
            ql = stile([P, TG], F32, "alo", 7)
            nc.scalar.copy(ql[:], pa[:])
            qlo.append(ql)

        # ---- RMS stats q ----
        ss_q = ptile([1, TG], *ACC)
        for m in range(QLC):
            sq = stile([P, TG], BF16, "sq", 1)
            nc.scalar.square(sq[:], qlo[m][:])
            nc.tensor.matmul(ss_q[:], ones_l[:], sq[:],
                             start=(m == 0), stop=(m == QLC - 1))
        rsq_q = stile([1, TG], F32, "rsq", 1)
        nc.scalar.activation(rsq_q[:], ss_q[:], mybir.ActivationFunctionType.Sqrt,
                             bias=eps_t[:], scale=1.0 / Q_LORA)
        bcq = stile([P, TG], F32, "bc", 2)
        if g == 0:
            inv_qr = stile([1, TG], F32R, "invr", 1)
            with nc.allow_low_precision(reason="fp32r inv scale"):
                nc.vector.reciprocal(inv_qr[:], rsq_q[:])
            bcq_p = ptile([P, TG], *MM)
            nc.tensor.matmul(bcq_p[:], ones_r[:], inv_qr[:], start=True, stop=True)
            nc.scalar.copy(bcq[:], bcq_p[:])
        else:
            inv_q = stile([1, TG], F32, "inv", 2)
            nc.vector.reciprocal(inv_q[:], rsq_q[:])
            nc.gpsimd.partition_broadcast(bcq[:], inv_q[:])

        qrms = []
        for m in range(QLC):
            qt = stile([P, TG], BF16, "qrms", QLC + 1)
            nc.vector.tensor_mul(qt[:], qlo[m][:], bcq[:])
            qrms.append(qt)

        # ---- A-proj: kv side ----
        kvlo = []
        for m in range(KVC):
            pa = ptile([P, TG], *PA)
            for k in range(KD):
                nc.tensor.matmul(pa[:], wkva_t[k][:, m * P:(m + 1) * P], hid[k][:],
                                 start=(k == 0), stop=(k == KD - 1))
            kl = stile([P, TG], F32, "alo", 7)
            nc.scalar.copy(kl[:], pa[:])
            kvlo.append(kl)
        # rope chunk [64, TG]
        krp = ptile([ROPE, TG], *PA)
        for k in range(KD):
            nc.tensor.matmul(krp[:], wkva_t[k][:, KV_RANK:KV_RANK + ROPE], hid[k][:],
                             start=(k == 0), stop=(k == KD - 1))
        kro = stile([ROPE, TG], F32, "kro", 1)
        nc.scalar.copy(kro[:], krp[:])
        nc.sync.dma_start(kro_out[:, gsl], kro[:])
        ck_g, sk_g = load_tab("ck", g), load_tab("sk", g)
        rope_rotate(krp[:], ck_g, sk_g, krotT[:, gsl])

        # ---- RMS stats kv ----
        ss_kv = ptile([1, TG], *ACC)
        for m in range(KVC):
            sq = stile([P, TG], BF16, "sq", 1)
            nc.scalar.square(sq[:], kvlo[m][:])
            nc.tensor.matmul(ss_kv[:], ones_l[:], sq[:],
                             start=(m == 0), stop=(m == KVC - 1))
        rsq_kv = stile([1, TG], F32, "rsq", 1)
        nc.scalar.activation(rsq_kv[:], ss_kv[:], mybir.ActivationFunctionType.Sqrt,
                             bias=eps_t[:], scale=1.0 / KV_RANK)
        bckv = stile([P, TG], F32, "bc", 2)
        if g == 0:
            inv_kvr = stile([1, TG], F32R, "invr", 1)
            with nc.allow_low_precision(reason="fp32r inv scale"):
                nc.vector.reciprocal(inv_kvr[:], rsq_kv[:])
            bckv_p = ptile([P, TG], *MM)
            nc.tensor.matmul(bckv_p[:], ones_r[:], inv_kvr[:], start=True, stop=True)
            nc.scalar.copy(bckv[:], bckv_p[:])
        else:
            inv_kv = stile([1, TG], F32, "inv", 2)
            nc.vector.reciprocal(inv_kv[:], rsq_kv[:])
            nc.gpsimd.partition_broadcast(bckv[:], inv_kv[:])

        # ---- c_kv^T = rms(kvc)*g : bf16 for compute + fp32 for output ----
        ckvg = []
        for m in range(KVC):
            cb = stile([P, TG], BF16, "ckvg", KVC)
            nc.vector.scalar_tensor_tensor(
                cb[:], kvlo[m][:], gk[m][:], bckv[:],
                op0=mybir.AluOpType.mult, op1=mybir.AluOpType.mult)
            ckvg.append(cb)
            co = stile([P, TG], F32, "ckvo", 1)
            nc.vector.scalar_tensor_tensor(
                co[:], kvlo[m][:], gk[m][:], bckv[:],
                op0=mybir.AluOpType.mult, op1=mybir.AluOpType.mult)
            nc.sync.dma_start(ckv_out[m * P:(m + 1) * P, gsl], co[:])

        # ---- q_b projection for this group ----
        qnT = []
        for m in range(HL):
            pq = ptile([P, TG], *MM)
            for k in range(QLC):
                nc.tensor.matmul(pq[:], wqbn_t[k][:, m * P:(m + 1) * P], qrms[k][:],
                                 start=(k == 0), stop=(k == QLC - 1))
            qt = stile([P, TG], BF16, "qnT", HL + 1)
            nc.scalar.copy(qt[:], pq[:])
            qnT.append(qt)
        qrT = []
        cq_g, sq_g = load_tab("cq", g), load_tab("sq", g)
        for mr in range(2):   # two rope chunks, 2 heads each
            pq = ptile([P, TG], *MM)
            for k in range(QLC):
                nc.tensor.matmul(pq[:], wqbr_t[k][:, mr * P:(mr + 1) * P], qrms[k][:],
                                 start=(k == 0), stop=(k == QLC - 1))
            for half in range(2):
                qt = stile([ROPE, TG], BF16, "qrT", 3)
                rope_rotate(pq[half * ROPE:(half + 1) * ROPE, :],
                            cq_g, sq_g, qt[:])
                qrT.append(qt)

        # ---- kv_b projection for this group ----
        for h in range(HL):
            pk = ptile([P, TG], *MM)
            for k in range(KVC):
                nc.tensor.matmul(pk[:], wkvbn_t[k][:, h * P:(h + 1) * P], ckvg[k][:],
                                 start=(k == 0), stop=(k == KVC - 1))
            nc.scalar.copy(knT[h][:, gsl], pk[:])
        for t in range(TG // P):
            pv = ptile([P, HL * V_DIM], *MM)
            for k in range(KVC):
                nc.tensor.matmul(pv[:], ckvg[k][:, t * P:(t + 1) * P], wkvbv_t[k][:],
                                 start=(k == 0), stop=(k == KVC - 1))
            nc.scalar.copy(V[g * (TG // P) + t][:], pv[:])

        # ---- attention: this group's queries, all 4 local heads ----
        onorm = []
        for h in range(HL):
            jmax = 4 * g + 3
            l_p = ptile([1, TG], *ACC)
            o_p = ptile([P, TG], *ACC)
            for j in range(jmax + 1):
                off = max(0, (j - 4 * g) * P)   # masked q-columns before off
                osl = slice(off, TG)
                s_p = ptile([P, TG], *MM)
                nc.tensor.matmul(s_p[:, osl], knT[h][:, j * P:(j + 1) * P],
                                 qnT[h][:, osl], start=True, stop=False)
                nc.tensor.matmul(s_p[:, osl], krotT[:, j * P:(j + 1) * P],
                                 qrT[h][:, osl], start=False, stop=True)
                pt = stile([P, TG], BF16, "PT", 4)
                nc.scalar.activation(pt[:, osl], s_p[:, osl],
                                     mybir.ActivationFunctionType.Exp)
                if j > 4 * g:
                    nc.vector.tensor_mul(pt[:, osl], pt[:, osl],
                                         masks[j - 4 * g][:, osl])
                elif j == 4 * g:
                    nc.vector.tensor_mul(pt[:], pt[:], masks[0][:])
                nc.tensor.matmul(l_p[:, osl], ones_l[:], pt[:, osl],
                                 start=(j == 0), stop=(j == jmax),
                                 skip_group_check=True)
                nc.tensor.matmul(o_p[:, osl], V[j][:, h * V_DIM:(h + 1) * V_DIM],
                                 pt[:, osl], start=(j == 0), stop=(j == jmax),
                                 skip_group_check=True)
            invl = stile([1, TG], F32, "inv", 2)
            nc.vector.reciprocal(invl[:], l_p[:])
            bcl = stile([P, TG], F32, "bc", 2)
            nc.gpsimd.partition_broadcast(bcl[:], invl[:])
            ot = stile([P, TG], BF16, "onorm", HL + 1)
            nc.vector.tensor_mul(ot[:], o_p[:], bcl[:])
            onorm.append(ot)

        # ---- o_proj (partial over local heads) for this group's tokens ----
        for dn in range(D // TG):
            for t in range(TG // P):
                po = ptile([P, TG], *MM)
                for h in range(HL):
                    nc.tensor.matmul(po[:], onorm[h][:, t * P:(t + 1) * P],
                                     wo_t[h][:, dn * TG:(dn + 1) * TG],
                                     start=(h == 0), stop=(h == HL - 1))
                oe = stile([P, TG], F32, "oe", 3)
                if g == NG - 1:
                    nc.scalar.copy(oe[:], po[:])
                else:
                    nc.vector.tensor_copy(oe[:], po[:])
                row = (g * (TG // P) + t) * P
                eng = nc.sync if (dn + t) % 2 == 0 else nc.scalar
                eng.dma_start(o_out[row:row + P, dn * TG:(dn + 1) * TG], oe[:])


def _build_program():
    nc = bacc.Bacc("TRN2", target_bir_lowering=False, debug=False, num_devices=8)
    io = (
        nc.dram_tensor("hT", [D, T], BF16, kind="ExternalInput").ap(),
        nc.dram_tensor("wqa", [D, Q_LORA], BF16, kind="ExternalInput").ap(),
        nc.dram_tensor("wkva", [D, KV_RANK + ROPE], BF16, kind="ExternalInput").ap(),
        nc.dram_tensor("wqbn", [Q_LORA, HL * NOPE], BF16, kind="ExternalInput").ap(),
        nc.dram_tensor("wqbr", [Q_LORA, HL * ROPE], BF16, kind="ExternalInput").ap(),
        nc.dram_tensor("wkvbn", [KV_RANK, HL * NOPE], BF16, kind="ExternalInput").ap(),
        nc.dram_tensor("wkvbv", [KV_RANK, HL * V_DIM], BF16, kind="ExternalInput").ap(),
        nc.dram_tensor("wo", [HL * V_DIM, D], BF16, kind="ExternalInput").ap(),
        nc.dram_tensor("gk", [KV_RANK, 1], F32, kind="ExternalInput").ap(),
        nc.dram_tensor("cosq", [ROPE, T], BF16, kind="ExternalInput").ap(),
        nc.dram_tensor("sinq", [ROPE, T], BF16, kind="ExternalInput").ap(),
        nc.dram_tensor("cosk", [ROPE, T], BF16, kind="ExternalInput").ap(),
        nc.dram_tensor("sink", [ROPE, T], BF16, kind="ExternalInput").ap(),
        nc.dram_tensor("masks", [4, P, TG], BF16, kind="ExternalInput").ap(),
        nc.dram_tensor("ones_r", [1, P], F32R, kind="ExternalInput").ap(),
        nc.dram_tensor("o_part", [T, D], F32, kind="ExternalOutput").ap(),
        nc.dram_tensor("ckv_T", [KV_RANK, T], F32, kind="ExternalOutput").ap(),
        nc.dram_tensor("kro_T", [ROPE, T], F32, kind="ExternalOutput").ap(),
    )
    with tile.TileContext(nc) as tc:
        _emit(nc, tc, io)
    nc.compile()
    return nc


def _get_program():
    global _PROG
    if _PROG is None:
        _PROG = _build_program()
    return _PROG


def _rope_tables_np(pos):
    """pos: int array [T] -> (cos^T, sinN^T) [64, T] fp32, sin sign-folded."""
    inv_freq = (1.0 / (10000.0 ** (np.arange(0, ROPE, 2, dtype=np.float32) / ROPE)))
    freqs = pos.astype(np.float32)[:, None] * inv_freq[None, :].astype(np.float32)
    emb = np.concatenate([freqs, freqs], axis=-1)                      # [T, 64]
    cos = np.cos(emb).astype(np.float32)
    sin = np.sin(emb).astype(np.float32)
    sinN = sin.copy()
    sinN[:, :RH] = -sinN[:, :RH]
    return np.ascontiguousarray(cos.T), np.ascontiguousarray(sinN.T)


def _prepare_in_maps(inputs):
    hidden = np.asarray(inputs["hidden_states"], dtype=np.float32)
    positions = np.asarray(inputs["positions"])
    w_qa = np.asarray(inputs["w_qa"], dtype=np.float32)
    g_qa = np.asarray(inputs["g_qa"], dtype=np.float32)
    w_qb = np.asarray(inputs["w_qb"], dtype=np.float32)
    w_kva = np.asarray(inputs["w_kva"], dtype=np.float32)
    g_kva = np.asarray(inputs["g_kva"], dtype=np.float32)
    w_kvb = np.asarray(inputs["w_kvb"], dtype=np.float32)
    w_o = np.asarray(inputs["w_o"], dtype=np.float32)

    bf = ml_dtypes.bfloat16
    # fold g_qa (rows) + attention scale into w_qb; split nope/rope per head
    wqb_f = (g_qa[:, None] * w_qb * np.float32(SCALE)).reshape(Q_LORA, H, Q_HEAD)
    wkvb_r = w_kvb.reshape(KV_RANK, H, NOPE + V_DIM)
    wo_r = w_o.reshape(H, V_DIM, D)

    # masks: mask_k[x, y] = 1 where y - x - 128k >= 0
    yy = np.arange(TG)[None, :]
    xx = np.arange(P)[:, None]
    masks = np.stack([(yy - xx - P * k >= 0) for k in range(4)]).astype(bf)

    cosk_t, sink_t = _rope_tables_np(np.arange(T))
    cosk_b, sink_b = cosk_t.astype(bf), sink_t.astype(bf)

    in_maps = []
    for c in range(8):
        b, hg = divmod(c, 4)
        hs = slice(hg * HL, (hg + 1) * HL)
        cosq_t, sinq_t = _rope_tables_np(np.asarray(positions[b]))
        in_maps.append({
            "hT": np.ascontiguousarray(hidden[b].T).astype(bf),
            "wqa": w_qa.astype(bf),
            "wkva": w_kva.astype(bf),
            "wqbn": np.ascontiguousarray(
                wqb_f[:, hs, :NOPE].reshape(Q_LORA, HL * NOPE)).astype(bf),
            "wqbr": np.ascontiguousarray(
                wqb_f[:, hs, NOPE:].reshape(Q_LORA, HL * ROPE)).astype(bf),
            "wkvbn": np.ascontiguousarray(
                wkvb_r[:, hs, :NOPE].reshape(KV_RANK, HL * NOPE)).astype(bf),
            "wkvbv": np.ascontiguousarray(
                wkvb_r[:, hs, NOPE:].reshape(KV_RANK, HL * V_DIM)).astype(bf),
            "wo": np.ascontiguousarray(wo_r[hs].reshape(HL * V_DIM, D)).astype(bf),
            "gk": g_kva.reshape(KV_RANK, 1),
            "cosq": cosq_t.astype(bf),
            "sinq": sinq_t.astype(bf),
            "cosk": cosk_b,
            "sink": sink_b,
            "masks": masks,
            "ones_r": np.ones((1, P), np.float32),
        })
    return in_maps


def kernel(**inputs):
    nc = _get_program()
    in_maps = _prepare_in_maps(inputs)
    res = run_bass_kernel_spmd(nc, in_maps, list(range(8))).results

    output = np.empty((B, T, D), np.float32)
    new_c_kv = np.empty((B, T, KV_RANK), np.float32)
    new_k_rope = np.empty((B, T, ROPE), np.float32)
    for b in range(B):
        output[b] = (res[4 * b]["o_part"] + res[4 * b + 1]["o_part"]
                     + res[4 * b + 2]["o_part"] + res[4 * b + 3]["o_part"])
        new_c_kv[b] = res[4 * b]["ckv_T"].T
        new_k_rope[b] = res[4 * b]["kro_T"].T
    return output, new_c_kv, new_k_rope
